# revision 8
# baseline (speedup 1.0000x reference)
"""Trainium2 Bass kernel for nn_DiagonalLinear: out = x * diagonal.

x: [8, 4096, 4096] f32, diagonal: [4096] f32.
Data-parallel over 8 NeuronCores: core i handles batch element i
([4096, 4096], 64 MiB). The [4096] diagonal is replicated to every core
and broadcast across the 128 SBUF partitions once; each row-tile of x is
DMA'd in, multiplied in place on the vector engine, and DMA'd back out.
Memory-bound: ~128 MiB of HBM traffic per core.
"""

import numpy as np

import concourse.bass as bass
import concourse.bacc as bacc
import concourse.tile as tile
from concourse import mybir
from concourse.bass_utils import run_bass_kernel_spmd

B, S, C = 8, 4096, 4096
P = 128  # SBUF partitions
N_CORES = 8

# Rows of x folded into each SBUF tile's free dim. ROWS_PER_TILE=2 gives
# 4 MiB per DMA (~90% DMA efficiency per the measured curve) at
# 32 KiB/partition/tile; 5 bufs keeps loads/compute/stores overlapped
# within the ~192 KiB/partition SBUF budget.
ROWS_PER_TILE = 2

_NC_CACHE: dict = {}


def _build_bass():
    """Per-core program: out[s, c] = x[s, c] * diagonal[c] for a
    [S, C] shard."""
    # Bacc (not raw Bass): its finalize() runs generate_event_semaphores,
    # which splits multi-sem waits (TRN2 allows one wait per instruction).
    nc = bacc.Bacc(None)
    x = nc.declare_dram_parameter("x", [S, C], mybir.dt.float32, isOutput=False)
    diag = nc.declare_dram_parameter(
        "diagonal", [C], mybir.dt.float32, isOutput=False
    )
    out = nc.declare_dram_parameter("out", [S, C], mybir.dt.float32, isOutput=True)

    # Partition p owns rows [p*R, (p+1)*R): each partition's span of x is
    # R*C*4 bytes of contiguous DRAM, so every DMA moves long sequential
    # chunks. Tile j covers rows p*R + [j*ROWS_PER_TILE, ...) per partition.
    R = S // P  # rows of x per partition
    n_tiles = R // ROWS_PER_TILE
    xv = x[:].rearrange("(p r) c -> p (r c)", p=P)
    ov = out[:].rearrange("(p r) c -> p (r c)", p=P)
    tile_w = ROWS_PER_TILE * C

    with tile.TileContext(nc) as tc:
        with (
            tc.tile_pool(name="const", bufs=1) as cpool,
            tc.tile_pool(name="work", bufs=4) as wpool,
        ):
            # Broadcast diagonal across all 128 partitions and repeat it
            # ROWS_PER_TILE times along the free dim (stride-0 DMA reads)
            # so the multiply below stays 2-D — a 3-D operand AP leaves
            # the TensorTensor ISA struct without enough sync-wait slots.
            dtile = cpool.tile([P, tile_w], mybir.dt.float32)
            dsrc = bass.AP(
                tensor=diag[:].tensor,
                offset=0,
                ap=[[0, P], [0, ROWS_PER_TILE], [1, C]],
            )
            nc.gpsimd.dma_start(out=dtile, in_=dsrc)

            for j in range(n_tiles):
                xt = wpool.tile([P, tile_w], mybir.dt.float32)
                nc.sync.dma_start(
                    out=xt, in_=xv[:, j * tile_w : (j + 1) * tile_w]
                )
                nc.vector.tensor_mul(out=xt, in0=xt, in1=dtile)
                nc.sync.dma_start(
                    out=ov[:, j * tile_w : (j + 1) * tile_w], in_=xt
                )
    nc.finalize()
    return nc


def _get_nc():
    if "nc" not in _NC_CACHE:
        _NC_CACHE["nc"] = _build_bass()
    return _NC_CACHE["nc"]


def _run(x, diagonal, **spmd_kwargs):
    x = np.ascontiguousarray(np.asarray(x, dtype=np.float32))
    diagonal = np.ascontiguousarray(np.asarray(diagonal, dtype=np.float32))
    assert x.shape == (B, S, C), x.shape
    assert diagonal.shape == (C,), diagonal.shape

    nc = _get_nc()
    in_maps = [{"x": x[i], "diagonal": diagonal} for i in range(N_CORES)]
    res = run_bass_kernel_spmd(nc, in_maps, list(range(N_CORES)), **spmd_kwargs)
    out = np.stack([res.results[i]["out"] for i in range(N_CORES)], axis=0)
    return out, res


def kernel(x, diagonal):
    out, _ = _run(x, diagonal)
    return out
